# revision 7
# baseline (speedup 1.0000x reference)
"""GCC-PHAT kernel for Trainium2, 8 NeuronCores, data-parallel over batch.

Input : x [128, 12, 4096] f32 -> out [128, 12, 12, 257] f32.
Each core takes 16 batch frames and computes their full 12x12 GCC-PHAT
independently (no collectives).

Per core:
  rfft(4096) as a 32x128 Cooley-Tukey: stage 1 (32-point DFTs on the PE)
  exploits the real input so only a packed 32-col half-spectrum per DFT
  is staged to SBUF (re q=0..16 | im q=1..15); stage 2 folds the
  conjugate bins into its twiddle/DFT weight blocks (4 t-blocks per q).
  PHAT normalize: 1/|X| = exp(-0.5 ln(|X|^2 + 1e-6)) on ACT (one table
  serves Copy/Square/Ln/Exp; stage-2 weights carry a 1/64 scale that
  cancels here). Cross-power for the 66 unordered pairs in a
  DIAGONAL-major row layout (pair (n, n+d) at OFF[d] + 16n + b), so
  every product is one slice-aligned elementwise mul - no broadcasts:
  m1 = a_n*a_m, m2 = b_n*b_m, m3 = (a+b)_n*(a-b)_m (Karatsuba), plus
  sg = m1+m2 on DVE. The lag-restricted inverse DFT accumulates in
  PSUM on the PE as 4 streams per j-group: eps += cos*sg and
  ops += -sin*m1 + sin*m2 + sin*m3 (Gim = m3 - m1 + m2), with
  out[+l] = eps - ops, out[-l] = eps + ops, lag 0 via per-j c0 columns,
  and the Nyquist bin handled by host-computed signs (cng).

  The 16 j-groups run as granules (2,2,4,4,4) in a software pipeline:
  stage2 two granules ahead, PHAT one ahead, so the in-order engine
  queues never serialize next-granule prep behind the product burst.
  Pool owns the big u3 diagonals (off the eps critical path); work is
  balanced DVE/Pool/ACT per the TimelineSim cost model.

Hardware notes: matmul start=True resets the whole PSUM bank row, so
each accumulator bank gets exactly one start and the shared small bank
(eps2|ops2|lag0) is DVE-memset instead. TensorTensor may read only one
input from PSUM (eps staged to SBUF via ACT before the final combine).

Self-contained: hardcodes shapes; only needs /opt/trn_rl_repo on sys.path.
"""
import os
import sys

sys.path.insert(0, "/opt/trn_rl_repo")

import numpy as np

B = 16            # batches per core
NSIG = 12
K = 4096
TAU = 128
NCORES = 8
NS = B * NSIG     # 192 signals per core
ROWS = 1056       # 66 pairs * 16 batches
# (j0, width) granules: small first two so the pipeline fills fast
GRANULES = [(0, 2), (2, 2), (4, 4), (8, 4), (12, 4)]
G_J = 4           # max granule width (tile sizing)

S2SCALE = 1.0 / 64.0

# diagonal-major pair rows: r = OFF[d] + n*16 + b  <-> pair (n, n+d)
OFF = {}
_o = 0
for _d in range(1, 12):
    OFF[_d] = _o
    _o += (12 - _d) * B
assert _o == ROWS

# chunks (PSUM banks): whole diagonals per chunk, <=512 f32 cols
CHUNK_DIAGS = [(2, (9, 10, 11)), (0, (1, 2, 3)), (1, (4, 5, 6, 7, 8))]
CHUNKS = {0: (0, 480), 1: (480, 480), 2: (960, 96)}
NBLK = (ROWS + 127) // 128       # 9 lag-0 col blocks

_COMPILED = {}
OUTPUT_NAMES = ["outp", "outm", "out0"]


def _fmap():
    """f(p, j): frequency of partition p in j-group j."""
    p = np.arange(128)
    j = np.arange(16)
    par = (p[:, None] >= 64).astype(np.int64)
    k2 = np.where(p[:, None] < 64, p[:, None], p[:, None] - 64)
    return (2 * j[None, :] + par) + 32 * k2   # [128, 16]


def _build_weights():
    f16 = np.float16
    # stage1 inputs are real, so bins q>16 are conjugates: emit a packed
    # 32-col spectrum per 32-DFT: cols 0..16 = re[q], 17..31 = im[q-16]
    n1 = np.arange(32)[:, None]
    q = np.arange(17)[None, :]
    qi = np.arange(1, 16)[None, :]
    w1_single = np.concatenate(
        [np.cos(2 * np.pi * n1 * q / 32.0),
         -np.sin(2 * np.pi * n1 * qi / 32.0)], axis=1)  # [32, 32]

    # stage2: w2d [128 n2, (qv 32, t 4, k2 64)]; applied to the PACKED
    # stage1 cols (are', aim') at qq = min(qv, 32-qv):
    #   re = t0*are' + t1*aim';  im = t2*are' + t3*aim'
    # For qv > 16 the conjugate flips aim: t1 = -sin, t3 = -cos there.
    n2 = np.arange(128)[:, None]
    k2 = np.arange(64)[None, :]
    w2 = np.zeros((128, 32, 4, 64), dtype=np.float64)
    for qv in range(32):
        ang = 2 * np.pi * (qv * n2 / 4096.0 + n2 * k2 / 128.0)
        sgn = 1.0 if qv <= 16 else -1.0
        w2[:, qv, 0, :] = np.cos(ang)
        w2[:, qv, 1, :] = sgn * np.sin(ang)
        w2[:, qv, 2, :] = -np.sin(ang)
        w2[:, qv, 3, :] = sgn * np.cos(ang)
    w2d = (w2 * S2SCALE).reshape(128, 32 * 4 * 64).astype(f16)

    fm = _fmap()                                    # [128, 16]
    cf = np.where(fm == 0, 1.0, 2.0) / K            # [128, 16]
    ll = np.arange(1, 129)
    # wcs [128, (j 16, cs 3 {cos, sin, -sin}, l 128)]
    wcs = np.zeros((128, 6144), dtype=f16)
    for j in range(16):
        ang = 2 * np.pi * fm[:, j:j + 1] * ll[None, :] / K   # [128, 128]
        base = 384 * j
        wc = (cf[:, j:j + 1] * np.cos(ang)).astype(f16)
        ws = (cf[:, j:j + 1] * np.sin(ang)).astype(f16)
        wcs[:, base:base + 128] = wc
        wcs[:, base + 128:base + 256] = ws
        wcs[:, base + 256:base + 384] = -ws

    # wmisc [128, 81]: cols 0:64 = w1 at partition bases 0/32/64/96,
    # col 64 unused, cols 65:81 = c0 (cf) per j
    wmisc = np.zeros((128, 81), dtype=f16)
    for g in range(4):
        wmisc[32 * g:32 * (g + 1), 0:32] = w1_single.astype(f16)
    wmisc[:, 65:81] = cf.astype(f16)

    # cng [1, 129]: nyquist weights: cols 0:128 = (1/K)(-1)^l l=1..128,
    # col 128 = 1/K (lag-0); g2048 appended at runtime per core
    cno = np.zeros(129, dtype=f16)
    cno[0:128] = ((1.0 / K) * ((-1.0) ** np.arange(1, 129))).astype(f16)
    cno[128] = np.float16(1.0 / K)
    return dict(wmisc=wmisc, w2d=w2d, wcs=wcs, cno=cno)


def _legalize_waits(nc):
    """This container's walrus accepts only ONE sync-wait per instruction.
    Split extra waits into single-wait NoOps inserted before, same engine."""
    from concourse import mybir
    nsplit = 0
    for b in nc.main_func.blocks:
        newlist = []
        for ins in b.instructions:
            si = ins.sync_info
            if si is not None and len(si.on_wait) > 1:
                waits = list(si.on_wait)
                for k, wt in enumerate(waits[:-1]):
                    nop = mybir.InstNoOp(name=f"{ins.name}-lw{k}", ins=[], outs=[])
                    nop.engine = ins.engine
                    nop.sync_info = mybir.SyncInfo(on_wait=[wt], on_update=[])
                    newlist.append(nop)
                    nsplit += 1
                ins.sync_info = mybir.SyncInfo(on_wait=[waits[-1]],
                                               on_update=list(si.on_update))
            newlist.append(ins)
        b.instructions = newlist
    return nsplit


def _build_bass():
    from concourse import bass, mybir, tile

    f32 = mybir.dt.float32
    f16 = mybir.dt.float16
    AF = mybir.ActivationFunctionType

    nc = bass.Bass()
    xd = nc.declare_dram_parameter("x", [128, 48 * 128], f16, isOutput=False)
    wmiscd = nc.declare_dram_parameter("wmisc", [128, 81], f16, isOutput=False)
    w2d = nc.declare_dram_parameter("w2d", [128, 32 * 4 * 64], f16,
                                    isOutput=False)
    wcsd = nc.declare_dram_parameter("wcs", [128, 6144], f16, isOutput=False)
    cngd = nc.declare_dram_parameter("cng", [1, 129 + ROWS], f16,
                                     isOutput=False)

    outpd = nc.declare_dram_parameter("outp", [128, ROWS], f16, isOutput=True)
    outmd = nc.declare_dram_parameter("outm", [128, ROWS], f16, isOutput=True)
    out0d = nc.declare_dram_parameter("out0", [128, NBLK], f32, isOutput=True)

    started_banks = set()

    def acc_mm(bank, out, lhsT, rhs, stop=False, tile_position=None):
        st = bank not in started_banks
        if st:
            started_banks.add(bank)
        kw = {}
        if tile_position is not None:
            kw["tile_position"] = tile_position
        nc.tensor.matmul(out, lhsT, rhs, start=st, stop=stop,
                         skip_group_check=True, **kw)

    with tile.TileContext(nc) as tc:
        with tc.tile_pool(name="const", bufs=1) as cpool:
            wmisc = cpool.tile([128, 81], f16, tag="wmisc")
            wcs = cpool.tile([128, 6144], f16, tag="wcs")
            cng = cpool.tile([1, 129 + ROWS], f16, tag="cng")
            w2sb = cpool.tile([128, 32 * 4 * 64], f16, tag="w2sb")
            bias = cpool.tile([128, 1], f32, tag="bias")
            AT = cpool.tile([128, 24 * 256], f16, tag="AT")
            outpsb = cpool.tile([128, ROWS], f16, tag="outpsb")
            outmsb = cpool.tile([128, ROWS], f16, tag="outmsb")
            out0sb = cpool.tile([128, NBLK], f32, tag="out0sb")
            nc.gpsimd.memset(bias[:], 1e-6)

            xin_scope = tc.tile_pool(name="xinp", bufs=1)
            xinp = xin_scope.__enter__()
            xin = xinp.tile([128, 48 * 128], f16, tag="xin")

            # ---- DMAs, in HWDGE order (JIT by first use) ----
            def dma(dst, src):
                nc.sync.dma_start(out=dst, in_=src)

            dma(xin[:, 0:1024], xd[:, 0:1024])
            dma(xin[:, 1024:2560], xd[:, 1024:2560])
            dma(wmisc[:], wmiscd[:])
            dma(xin[:, 2560:4096], xd[:, 2560:4096])
            dma(w2sb[:, 0:1024], w2d[:, 0:1024])
            dma(wcs[:, 0:768], wcsd[:, 0:768])
            dma(xin[:, 4096:5632], xd[:, 4096:5632])
            dma(xin[:, 5632:6144], xd[:, 5632:6144])
            dma(w2sb[:, 1024:2048], w2d[:, 1024:2048])
            dma(wcs[:, 768:1536], wcsd[:, 768:1536])
            for j0, gw in GRANULES[2:]:
                dma(w2sb[:, 512 * j0:512 * (j0 + gw)],
                    w2d[:, 512 * j0:512 * (j0 + gw)])
                dma(wcs[:, 384 * j0:384 * (j0 + gw)],
                    wcsd[:, 384 * j0:384 * (j0 + gw)])
            dma(cng[:], cngd[:])

            # ---- stage 1 (gp-major: AT blocks 0-11 complete early) ----
            # accumulator banks are allocated AFTER stage1 so psA can use
            # 6 banks: the ps->AT copies never stall the stage1 matmuls
            wdump = cpool.tile([128, 1], f16, tag="wdump")
            with tc.tile_pool(name="psA", bufs=6, space="PSUM") as psA:
                # PE warmup while the first xin chunk is in flight: keeps
                # the p-state ramp going so stage1 starts at speed
                warm = cpool.tile([128, 512], f16, tag="warm")
                nc.gpsimd.memset(warm[:], 0.0)
                psw = psA.tile([128, 512], f32, tag="s1", name="psw")
                for wi in range(8):
                    nc.tensor.matmul(psw[:, 0:512], warm[0:32, 0:128],
                                     warm[0:32, 0:512],
                                     start=(wi == 0), stop=(wi == 7),
                                     skip_group_check=True)
                nc.scalar.copy(wdump[:], psw[:, 0:1])
                it = 0
                for rp in range(3):
                    for gp in range(2):
                        for gh in range(2):
                            g = 2 * gp + gh
                            ps = psA.tile([128, 512], f32, tag="s1")
                            for rr in range(2):
                                rblk = 2 * rp + rr
                                xt = xin[32 * g:32 * (g + 1),
                                         1024 * rblk:1024 * (rblk + 1)]
                                for sp in range(8):
                                    nc.tensor.matmul(
                                        ps[:, 256 * rr + 32 * sp:
                                           256 * rr + 32 * (sp + 1)],
                                        xt[:, 128 * sp:128 * (sp + 1)],
                                        wmisc[32 * g:32 * (g + 1), 0:32],
                                        start=True, stop=True,
                                        tile_position=(32 * g, 0))
                            blk = 12 * gp + 6 * gh + 2 * rp
                            dst = AT[:, 256 * blk:256 * (blk + 2)]
                            if it % 2 == 1:
                                nc.scalar.copy(dst, ps[:])
                            else:
                                nc.vector.tensor_copy(dst, ps[:])
                            it += 1

            atv = AT[:].rearrange("p (s c) -> p s c", s=192, c=32)
            w2v = w2sb[:].rearrange("p (q t k) -> p q t k", q=32, t=4, k=64)

            xin_scope.__exit__(None, None, None)

            # ---- PSUM accumulators (5 banks) ----
            acc_scope = tc.tile_pool(name="acc", bufs=1, space="PSUM")
            accp = acc_scope.__enter__()
            eps0 = accp.tile([128, 480], f32, tag="eps0", name="eps0")
            ops0 = accp.tile([128, 480], f32, tag="ops0", name="ops0")
            eps1 = accp.tile([128, 480], f32, tag="eps1", name="eps1")
            ops1 = accp.tile([128, 480], f32, tag="ops1", name="ops1")
            small = accp.tile([128, 208], f32, tag="small", name="small")
            eps_t = {0: eps0[:], 1: eps1[:], 2: small[:, 0:96]}
            ops_t = {0: ops0[:], 1: ops1[:], 2: small[:, 96:192]}
            zt = small[:, 192:192 + NBLK]
            # bank ids for the start-once bookkeeping; the shared small
            # bank is memset instead (start=True would reset the whole
            # bank and clobber its sibling regions)
            nc.vector.memset(small[:], 0.0)
            started_banks.add("sm")
            eps_bank = {0: "e0", 1: "e1", 2: "sm"}
            ops_bank = {0: "o0", 1: "o1", 2: "sm"}

            psX_scope = tc.tile_pool(name="psX", bufs=2, space="PSUM")
            psX = psX_scope.__enter__()

            with (
                tc.tile_pool(name="phX", bufs=3) as phX,
                tc.tile_pool(name="phT", bufs=3) as phT,
                tc.tile_pool(name="up", bufs=3) as up,
                tc.tile_pool(name="sgp", bufs=2) as sgp,
                tc.tile_pool(name="finp", bufs=2) as finp,
            ):
                def stage2(gi):
                    j0, gw = GRANULES[gi]
                    Xg = phX.tile([128, G_J * 384], f16, tag="Xg",
                                  name=f"Xg{gi}")
                    for jl in range(gw):
                        j = j0 + jl
                        x2 = psX.tile([128, 384], f32, tag="x2")
                        for par in range(2):
                            qv = 2 * j + par
                            qq = qv if qv <= 16 else 32 - qv
                            has_im = 0 < qq < 16
                            are = atv[:, :, qq]
                            re_out = x2[64 * par:64 * (par + 1), 0:192]
                            im_out = x2[64 * par:64 * (par + 1), 192:384]
                            aim = atv[:, :, 16 + qq] if has_im else None
                            nc.tensor.matmul(re_out, w2v[:, qv, 0, :], are,
                                             start=True, stop=not has_im)
                            if has_im:
                                nc.tensor.matmul(re_out, w2v[:, qv, 1, :],
                                                 aim, start=False, stop=True)
                            nc.tensor.matmul(im_out, w2v[:, qv, 2, :], are,
                                             start=True, stop=not has_im)
                            if has_im:
                                nc.tensor.matmul(im_out, w2v[:, qv, 3, :],
                                                 aim, start=False, stop=True)
                        nc.scalar.copy(Xg[:, 384 * jl:384 * (jl + 1)], x2[:])
                    return Xg

                def phat(gi, Xg):
                    j0, gw = GRANULES[gi]
                    xw = Xg[:, 0:gw * 384].rearrange(
                        "p (jl r s) -> p jl r s", jl=gw, r=2, s=192)
                    sq = phT.tile([128, G_J * 384], f16, tag="sq",
                                  name=f"sq{gi}")
                    mg = phT.tile([128, G_J * 192], f16, tag="mg",
                                  name=f"mg{gi}")
                    rb = phT.tile([128, G_J * 192], f16, tag="rb",
                                  name=f"rb{gi}")
                    Ct = phT.tile([128, G_J * 192], f16, tag="Ct",
                                  name=f"Ct{gi}")
                    Dt = phT.tile([128, G_J * 192], f16, tag="Dt",
                                  name=f"Dt{gi}")
                    sqv = sq[:, 0:gw * 384].rearrange(
                        "p (jl r s) -> p jl r s", jl=gw, r=2, s=192)
                    mgv = mg[:, 0:gw * 192].rearrange(
                        "p (jl s) -> p jl s", jl=gw, s=192)
                    nc.scalar.activation(sq[:, 0:gw * 384], Xg[:, 0:gw * 384],
                                         AF.Square)
                    nc.vector.tensor_add(mgv, sqv[:, :, 0, :], sqv[:, :, 1, :])
                    # 1/|X| = exp(-0.5 ln(mag2 + 1e-6)); bias guards f16
                    # underflow (Copy/Square/Ln/Exp share one ACT table)
                    lnt = phT.tile([128, G_J * 192], f16, tag="lnt",
                                   name=f"lnt{gi}")
                    nc.scalar.activation(lnt[:, 0:gw * 192], mg[:, 0:gw * 192],
                                         AF.Ln, bias=bias[:])
                    nc.scalar.activation(rb[:, 0:gw * 192], lnt[:, 0:gw * 192],
                                         AF.Exp, scale=-0.5)
                    rbb = rb[:, 0:gw * 192].rearrange(
                        "p (jl s) -> p jl s", jl=gw, s=192)
                    rbb = rbb.unsqueeze(2).broadcast_to((128, gw, 2, 192))
                    nc.vector.tensor_mul(xw, xw, rbb)
                    av = xw[:, :, 0, :]
                    bv = xw[:, :, 1, :]
                    Cv = Ct[:, 0:gw * 192].rearrange(
                        "p (jl s) -> p jl s", jl=gw, s=192)
                    Dv = Dt[:, 0:gw * 192].rearrange(
                        "p (jl s) -> p jl s", jl=gw, s=192)
                    cd_eng = nc.vector if gi < 2 else nc.gpsimd
                    cd_eng.tensor_add(Cv, av, bv)
                    cd_eng.tensor_sub(Dv, av, bv)
                    return av, bv, Cv, Dv

                POOL_D = (1, 2, 3, 4, 5)

                def zpass(j0, gw, sgv):
                    for jl in range(gw):
                        j = j0 + jl
                        c0col = wmisc[:, 65 + j:66 + j]
                        for bi in range(NBLK):
                            zr0 = 128 * bi
                            zw = min(128, ROWS - zr0)
                            acc_mm("sm", zt[0:zw, bi:bi + 1],
                                   sgv[:, jl, zr0:zr0 + zw], c0col)

                def granule(gi, ph, last):
                    j0, gw = GRANULES[gi]
                    av, bv, Cv, Dv = ph
                    u1 = up.tile([128, G_J * ROWS], f16, tag="u1",
                                 name=f"u1_{gi}")
                    u2 = up.tile([128, G_J * ROWS], f16, tag="u2",
                                 name=f"u2_{gi}")
                    u3 = up.tile([128, G_J * ROWS], f16, tag="u3",
                                 name=f"u3_{gi}")
                    sg = sgp.tile([128, G_J * ROWS], f16, tag="sg",
                                  name=f"sg_{gi}")
                    u1v = u1[:, 0:gw * ROWS].rearrange(
                        "p (jl r) -> p jl r", jl=gw, r=ROWS)
                    u2v = u2[:, 0:gw * ROWS].rearrange(
                        "p (jl r) -> p jl r", jl=gw, r=ROWS)
                    u3v = u3[:, 0:gw * ROWS].rearrange(
                        "p (jl r) -> p jl r", jl=gw, r=ROWS)
                    sgv = sg[:, 0:gw * ROWS].rearrange(
                        "p (jl r) -> p jl r", jl=gw, r=ROWS)

                    def mul_d(uv, lv, rv, d, pool_ok=False):
                        wd = (12 - d) * B
                        o = OFF[d]
                        eng = nc.gpsimd if (pool_ok and d in POOL_D) \
                            else nc.vector
                        eng.tensor_mul(uv[:, :, o:o + wd],
                                       lv[:, :, 0:wd],
                                       rv[:, :, d * B:d * B + wd])

                    def wslice(cs, jl):
                        b0 = 384 * (j0 + jl) + 128 * cs
                        return wcs[:, b0:b0 + 128]

                    def passes(ci, streams, stop_u3=False):
                        c0, cw = CHUNKS[ci]
                        for (kind, uv) in streams:
                            bank = eps_bank[ci] if kind == 0 else ops_bank[ci]
                            acc = eps_t[ci] if kind == 0 else ops_t[ci]
                            for jl in range(gw):
                                acc_mm(bank, acc, wslice(kind, jl),
                                       uv[:, jl, c0:c0 + cw],
                                       stop=(stop_u3 and kind == 1
                                             and uv is u3v
                                             and jl == gw - 1))

                    if not last:
                        # stream-major: muls u1, u2, sg, u3 then PE in the
                        # same production order
                        for d in range(1, 12):
                            mul_d(u1v, av, av, d, pool_ok=(d in U1_POOL))
                        for d in range(1, 12):
                            mul_d(u2v, bv, bv, d)
                        nc.vector.tensor_add(sg[:, 0:gw * ROWS],
                                             u1[:, 0:gw * ROWS],
                                             u2[:, 0:gw * ROWS])
                        for d in range(1, 12):
                            mul_d(u3v, Cv, Dv, d, pool_ok=True)
                        for ci in range(3):
                            passes(ci, [(2, u1v)])
                        for ci in range(3):
                            passes(ci, [(1, u3v)])
                        for ci in range(3):
                            passes(ci, [(1, u2v)])
                        for ci in range(3):
                            passes(ci, [(0, sgv)])
                        zpass(j0, gw, sgv)
                    else:
                        # chunk-major: each chunk finishes (incl nyquist,
                        # combine, DMA out) while the next chunk computes
                        for ci, dlist in [(0, (1, 2, 3)), (1, (4, 5, 6, 7, 8)),
                                          (2, (9, 10, 11))]:
                            c0, cw = CHUNKS[ci]
                            for d in dlist:
                                mul_d(u1v, av, av, d)
                            for d in dlist:
                                mul_d(u2v, bv, bv, d)
                            nc.vector.tensor_add(
                                sgv[:, :, c0:c0 + cw],
                                u1v[:, :, c0:c0 + cw],
                                u2v[:, :, c0:c0 + cw])
                            if ci == 2:
                                zpass(j0, gw, sgv)
                                finish_z()
                            for d in dlist:
                                mul_d(u3v, Cv, Dv, d, pool_ok=True)
                            passes(ci, [(2, u1v), (1, u2v), (0, sgv)])
                            passes(ci, [(1, u3v)], stop_u3=True)
                            finish_chunk(ci)

                def finish_chunk(ci):
                    c0, cw = CHUNKS[ci]
                    # nyquist term joins eps; full-width stop
                    acc_mm(eps_bank[ci], eps_t[ci],
                           cng[0:1, 0:128], cng[0:1, 129 + c0:129 + c0 + cw],
                           stop=True)
                    # outp = eps - ops (lags +1..+128), outm = eps + ops.
                    # TensorTensor may read only ONE input from PSUM:
                    # stage eps to SBUF (ACT) first
                    esb = finp.tile([128, 480], f32, tag="esb",
                                    name=f"esb{ci}")
                    nc.scalar.copy(esb[:, 0:cw], eps_t[ci])
                    nc.vector.tensor_sub(outpsb[:, c0:c0 + cw],
                                         esb[:, 0:cw], ops_t[ci])
                    nc.vector.tensor_add(outmsb[:, c0:c0 + cw],
                                         esb[:, 0:cw], ops_t[ci])
                    nc.sync.dma_start(out=outpd[:, c0:c0 + cw],
                                      in_=outpsb[:, c0:c0 + cw])
                    nc.sync.dma_start(out=outmd[:, c0:c0 + cw],
                                      in_=outmsb[:, c0:c0 + cw])

                def finish_z():
                    for bi in range(NBLK):
                        zr0 = 128 * bi
                        zw = min(128, ROWS - zr0)
                        acc_mm("sm", zt[0:zw, bi:bi + 1],
                               cng[0:1, 129 + zr0:129 + zr0 + zw],
                               cng[0:1, 128:129],
                               stop=(bi == NBLK - 1))
                    nc.vector.tensor_copy(out0sb[:], zt)
                    nc.sync.dma_start(out=out0d[:], in_=out0sb[:])

                # ---- software pipeline, stage2 two granules ahead and
                # phat one ahead: phat(g+1)'s DVE/ACT/Pool ops are issued
                # before muls(g) so the in-order queues can't serialize
                # next-granule prep behind this granule's product burst ----
                NG = len(GRANULES)
                Xgs = {0: stage2(0), 1: stage2(1)}
                phs = {0: phat(0, Xgs.pop(0))}
                for gi in range(NG):
                    if gi + 2 < NG:
                        Xgs[gi + 2] = stage2(gi + 2)
                    if gi + 1 < NG:
                        phs[gi + 1] = phat(gi + 1, Xgs.pop(gi + 1))
                    granule(gi, phs.pop(gi), last=(gi == NG - 1))

            psX_scope.__exit__(None, None, None)
            acc_scope.__exit__(None, None, None)

    _legalize_waits(nc)
    return nc


def pack_inputs_core(xs):
    """xs [16, 12, 4096] f32 -> input map for one core."""
    if "w" not in _COMPILED:
        _COMPILED["w"] = _build_weights()
    W = _COMPILED["w"]
    xdev = xs.astype(np.float16)
    xt1 = xdev.transpose(1, 0, 2).reshape(NS, 32, 128)   # s=(n,b), n1, n2
    xc = xt1.reshape(4, 48, 32, 128).transpose(0, 2, 1, 3) \
        .reshape(128, 48 * 128)
    alt = ((-1.0) ** np.arange(K)).astype(np.float32)
    xnyq = (xs.astype(np.float32) * alt).sum(axis=2)     # [16, 12]
    sgn = np.where(xnyq >= 0, 1.0, -1.0).T               # [12 n, 16 b]
    cng = np.zeros((1, 129 + ROWS), np.float16)
    cng[0, 0:129] = W["cno"]
    for d in range(1, 12):
        for n in range(12 - d):
            r = OFF[d] + n * B
            cng[0, 129 + r:129 + r + B] = \
                (sgn[n] * sgn[n + d]).astype(np.float16)
    return {"x": np.ascontiguousarray(xc), "cng": cng,
            "wmisc": W["wmisc"], "w2d": W["w2d"], "wcs": W["wcs"]}


def unpack_outputs_core(outs, xs):
    """outs dict -> [16, 12, 12, 257] f32."""
    outp = outs["outp"].astype(np.float32)
    outm = outs["outm"].astype(np.float32)
    out0 = outs["out0"]
    zz = np.zeros(ROWS, np.float32)
    for bi in range(NBLK):
        c0 = 128 * bi
        cw = min(128, ROWS - c0)
        zz[c0:c0 + cw] = out0[0:cw, bi]
    blk = np.zeros((B, NSIG, NSIG, 2 * TAU + 1), dtype=np.float32)
    for d in range(1, 12):
        for n in range(12 - d):
            m = n + d
            rows = OFF[d] + n * B + np.arange(B)
            blk[:, n, m, 0] = zz[rows]
            blk[:, n, m, 1:129] = outp[:, rows].T
            blk[:, n, m, 129:] = outm[::-1, rows].T
            blk[:, m, n, 0] = zz[rows]
            blk[:, m, n, 1:] = blk[:, n, m, 1:][:, ::-1]
    for n in range(NSIG):
        blk[:, n, n, 0] = 1.0
    return blk


def _get_compiled():
    if "nc" not in _COMPILED:
        _COMPILED["nc"] = _build_bass()
    return _COMPILED["nc"]


def kernel(x: np.ndarray) -> np.ndarray:
    from concourse.bass_utils import run_bass_kernel_spmd

    nc = _get_compiled()
    x = np.ascontiguousarray(x, dtype=np.float32)
    in_maps = [pack_inputs_core(x[c * B:(c + 1) * B]) for c in range(NCORES)]

    trace = bool(int(os.environ.get("BASS_GCC_TRACE", "0")))
    res = None
    for attempt in range(3):
        try:
            res = run_bass_kernel_spmd(nc, in_maps, list(range(NCORES)),
                                       trace=trace)
            break
        except Exception:
            if attempt == 2:
                raise
            import time
            time.sleep(5)
    _COMPILED["last_result"] = res

    out = np.zeros((NCORES * B, NSIG, NSIG, 2 * TAU + 1), dtype=np.float32)
    for c in range(NCORES):
        out[c * B:(c + 1) * B] = unpack_outputs_core(
            res.results[c], x[c * B:(c + 1) * B])
    return out


# revision 8
# speedup vs baseline: 1.0008x; 1.0008x over previous
"""GCC-PHAT kernel for Trainium2, 8 NeuronCores, data-parallel over batch.

Input : x [128, 12, 4096] f32 -> out [128, 12, 12, 257] f32.
Each core takes 16 batch frames and computes their full 12x12 GCC-PHAT
independently (no collectives).

Per core:
  rfft(4096) as a 32x128 Cooley-Tukey: stage 1 (32-point DFTs on the PE)
  exploits the real input so only a packed 32-col half-spectrum per DFT
  is staged to SBUF (re q=0..16 | im q=1..15); stage 2 folds the
  conjugate bins into its twiddle/DFT weight blocks (4 t-blocks per q).
  PHAT normalize: 1/|X| = exp(-0.5 ln(|X|^2 + 1e-6)) on ACT (one table
  serves Copy/Square/Ln/Exp; stage-2 weights carry a 1/64 scale that
  cancels here). Cross-power for the 66 unordered pairs in a
  DIAGONAL-major row layout (pair (n, n+d) at OFF[d] + 16n + b), so
  every product is one slice-aligned elementwise mul - no broadcasts:
  m1 = a_n*a_m, m2 = b_n*b_m, m3 = (a+b)_n*(a-b)_m (Karatsuba), plus
  sg = m1+m2 on DVE. The lag-restricted inverse DFT accumulates in
  PSUM on the PE as 4 streams per j-group: eps += cos*sg and
  ops += -sin*m1 + sin*m2 + sin*m3 (Gim = m3 - m1 + m2), with
  out[+l] = eps - ops, out[-l] = eps + ops, lag 0 via per-j c0 columns,
  and the Nyquist bin handled by host-computed signs (cng).

  The 16 j-groups run as granules (2,2,4,4,4) in a software pipeline:
  stage2 two granules ahead, PHAT one ahead, so the in-order engine
  queues never serialize next-granule prep behind the product burst.
  Pool owns the big u3 diagonals (off the eps critical path); work is
  balanced DVE/Pool/ACT per the TimelineSim cost model.

Hardware notes: matmul start=True resets the whole PSUM bank row, so
each accumulator bank gets exactly one start and the shared small bank
(eps2|ops2|lag0) is DVE-memset instead. TensorTensor may read only one
input from PSUM (eps staged to SBUF via ACT before the final combine).

Self-contained: hardcodes shapes; only needs /opt/trn_rl_repo on sys.path.
"""
import os
import sys

sys.path.insert(0, "/opt/trn_rl_repo")

import numpy as np

B = 16            # batches per core
NSIG = 12
K = 4096
TAU = 128
NCORES = 8
NS = B * NSIG     # 192 signals per core
ROWS = 1056       # 66 pairs * 16 batches
# (j0, width) granules: small first two so the pipeline fills fast
GRANULES = [(0, 2), (2, 2), (4, 4), (8, 4), (12, 4)]
G_J = 4           # max granule width (tile sizing)

S2SCALE = 1.0 / 64.0

# diagonal-major pair rows: r = OFF[d] + n*16 + b  <-> pair (n, n+d)
OFF = {}
_o = 0
for _d in range(1, 12):
    OFF[_d] = _o
    _o += (12 - _d) * B
assert _o == ROWS

# chunks (PSUM banks): whole diagonals per chunk, <=512 f32 cols
CHUNK_DIAGS = [(2, (9, 10, 11)), (0, (1, 2, 3)), (1, (4, 5, 6, 7, 8))]
CHUNKS = {0: (0, 480), 1: (480, 480), 2: (960, 96)}
NBLK = (ROWS + 127) // 128       # 9 lag-0 col blocks

_COMPILED = {}
OUTPUT_NAMES = ["outp", "outm", "out0"]


def _fmap():
    """f(p, j): frequency of partition p in j-group j."""
    p = np.arange(128)
    j = np.arange(16)
    par = (p[:, None] >= 64).astype(np.int64)
    k2 = np.where(p[:, None] < 64, p[:, None], p[:, None] - 64)
    return (2 * j[None, :] + par) + 32 * k2   # [128, 16]


def _build_weights():
    f16 = np.float16
    # stage1 inputs are real, so bins q>16 are conjugates: emit a packed
    # 32-col spectrum per 32-DFT: cols 0..16 = re[q], 17..31 = im[q-16]
    n1 = np.arange(32)[:, None]
    q = np.arange(17)[None, :]
    qi = np.arange(1, 16)[None, :]
    w1_single = np.concatenate(
        [np.cos(2 * np.pi * n1 * q / 32.0),
         -np.sin(2 * np.pi * n1 * qi / 32.0)], axis=1)  # [32, 32]

    # stage2: w2d [128 n2, (qv 32, t 4, k2 64)]; applied to the PACKED
    # stage1 cols (are', aim') at qq = min(qv, 32-qv):
    #   re = t0*are' + t1*aim';  im = t2*are' + t3*aim'
    # For qv > 16 the conjugate flips aim: t1 = -sin, t3 = -cos there.
    n2 = np.arange(128)[:, None]
    k2 = np.arange(64)[None, :]
    w2 = np.zeros((128, 32, 4, 64), dtype=np.float64)
    for qv in range(32):
        ang = 2 * np.pi * (qv * n2 / 4096.0 + n2 * k2 / 128.0)
        sgn = 1.0 if qv <= 16 else -1.0
        w2[:, qv, 0, :] = np.cos(ang)
        w2[:, qv, 1, :] = sgn * np.sin(ang)
        w2[:, qv, 2, :] = -np.sin(ang)
        w2[:, qv, 3, :] = sgn * np.cos(ang)
    w2d = (w2 * S2SCALE).reshape(128, 32 * 4 * 64).astype(f16)

    fm = _fmap()                                    # [128, 16]
    cf = np.where(fm == 0, 1.0, 2.0) / K            # [128, 16]
    ll = np.arange(1, 129)
    # wcs [128, (j 16, cs 3 {cos, sin, -sin}, l 128)]
    wcs = np.zeros((128, 6144), dtype=f16)
    for j in range(16):
        ang = 2 * np.pi * fm[:, j:j + 1] * ll[None, :] / K   # [128, 128]
        base = 384 * j
        wc = (cf[:, j:j + 1] * np.cos(ang)).astype(f16)
        ws = (cf[:, j:j + 1] * np.sin(ang)).astype(f16)
        wcs[:, base:base + 128] = wc
        wcs[:, base + 128:base + 256] = ws
        wcs[:, base + 256:base + 384] = -ws

    # wmisc [128, 81]: cols 0:64 = w1 at partition bases 0/32/64/96,
    # col 64 unused, cols 65:81 = c0 (cf) per j
    wmisc = np.zeros((128, 81), dtype=f16)
    for g in range(4):
        wmisc[32 * g:32 * (g + 1), 0:32] = w1_single.astype(f16)
    wmisc[:, 65:81] = cf.astype(f16)

    # cng [1, 129]: nyquist weights: cols 0:128 = (1/K)(-1)^l l=1..128,
    # col 128 = 1/K (lag-0); g2048 appended at runtime per core
    cno = np.zeros(129, dtype=f16)
    cno[0:128] = ((1.0 / K) * ((-1.0) ** np.arange(1, 129))).astype(f16)
    cno[128] = np.float16(1.0 / K)
    return dict(wmisc=wmisc, w2d=w2d, wcs=wcs, cno=cno)


def _legalize_waits(nc):
    """This container's walrus accepts only ONE sync-wait per instruction.
    Split extra waits into single-wait NoOps inserted before, same engine."""
    from concourse import mybir
    nsplit = 0
    for b in nc.main_func.blocks:
        newlist = []
        for ins in b.instructions:
            si = ins.sync_info
            if si is not None and len(si.on_wait) > 1:
                waits = list(si.on_wait)
                for k, wt in enumerate(waits[:-1]):
                    nop = mybir.InstNoOp(name=f"{ins.name}-lw{k}", ins=[], outs=[])
                    nop.engine = ins.engine
                    nop.sync_info = mybir.SyncInfo(on_wait=[wt], on_update=[])
                    newlist.append(nop)
                    nsplit += 1
                ins.sync_info = mybir.SyncInfo(on_wait=[waits[-1]],
                                               on_update=list(si.on_update))
            newlist.append(ins)
        b.instructions = newlist
    return nsplit


def _build_bass():
    from concourse import bass, mybir, tile

    f32 = mybir.dt.float32
    f16 = mybir.dt.float16
    AF = mybir.ActivationFunctionType

    nc = bass.Bass()
    xd = nc.declare_dram_parameter("x", [128, 48 * 128], f16, isOutput=False)
    wmiscd = nc.declare_dram_parameter("wmisc", [128, 81], f16, isOutput=False)
    w2d = nc.declare_dram_parameter("w2d", [128, 32 * 4 * 64], f16,
                                    isOutput=False)
    wcsd = nc.declare_dram_parameter("wcs", [128, 6144], f16, isOutput=False)
    cngd = nc.declare_dram_parameter("cng", [1, 129 + ROWS], f16,
                                     isOutput=False)

    outpd = nc.declare_dram_parameter("outp", [128, ROWS], f16, isOutput=True)
    outmd = nc.declare_dram_parameter("outm", [128, ROWS], f16, isOutput=True)
    out0d = nc.declare_dram_parameter("out0", [128, NBLK], f32, isOutput=True)

    started_banks = set()

    def acc_mm(bank, out, lhsT, rhs, stop=False, tile_position=None):
        st = bank not in started_banks
        if st:
            started_banks.add(bank)
        kw = {}
        if tile_position is not None:
            kw["tile_position"] = tile_position
        nc.tensor.matmul(out, lhsT, rhs, start=st, stop=stop,
                         skip_group_check=True, **kw)

    with tile.TileContext(nc) as tc:
        with tc.tile_pool(name="const", bufs=1) as cpool:
            wmisc = cpool.tile([128, 81], f16, tag="wmisc")
            wcs = cpool.tile([128, 6144], f16, tag="wcs")
            cng = cpool.tile([1, 129 + ROWS], f16, tag="cng")
            w2sb = cpool.tile([128, 32 * 4 * 64], f16, tag="w2sb")
            bias = cpool.tile([128, 1], f32, tag="bias")
            AT = cpool.tile([128, 24 * 256], f16, tag="AT")
            outpsb = cpool.tile([128, ROWS], f16, tag="outpsb")
            outmsb = cpool.tile([128, ROWS], f16, tag="outmsb")
            out0sb = cpool.tile([128, NBLK], f32, tag="out0sb")
            nc.gpsimd.memset(bias[:], 1e-6)

            xin_scope = tc.tile_pool(name="xinp", bufs=1)
            xinp = xin_scope.__enter__()
            xin = xinp.tile([128, 48 * 128], f16, tag="xin")

            # ---- DMAs, in HWDGE order (JIT by first use) ----
            def dma(dst, src):
                nc.sync.dma_start(out=dst, in_=src)

            dma(xin[:, 0:1024], xd[:, 0:1024])
            dma(xin[:, 1024:2560], xd[:, 1024:2560])
            dma(wmisc[:], wmiscd[:])
            dma(xin[:, 2560:4096], xd[:, 2560:4096])
            dma(w2sb[:, 0:1024], w2d[:, 0:1024])
            dma(wcs[:, 0:768], wcsd[:, 0:768])
            dma(xin[:, 4096:5632], xd[:, 4096:5632])
            dma(xin[:, 5632:6144], xd[:, 5632:6144])
            dma(w2sb[:, 1024:2048], w2d[:, 1024:2048])
            dma(wcs[:, 768:1536], wcsd[:, 768:1536])
            for j0, gw in GRANULES[2:]:
                dma(w2sb[:, 512 * j0:512 * (j0 + gw)],
                    w2d[:, 512 * j0:512 * (j0 + gw)])
                dma(wcs[:, 384 * j0:384 * (j0 + gw)],
                    wcsd[:, 384 * j0:384 * (j0 + gw)])
            dma(cng[:], cngd[:])

            # ---- stage 1 (gp-major: AT blocks 0-11 complete early) ----
            # accumulator banks are allocated AFTER stage1 so psA can use
            # 6 banks: the ps->AT copies never stall the stage1 matmuls
            wdump = cpool.tile([128, 1], f16, tag="wdump")
            with tc.tile_pool(name="psA", bufs=6, space="PSUM") as psA:
                # PE warmup while the first xin chunk is in flight: keeps
                # the p-state ramp going so stage1 starts at speed
                warm = cpool.tile([128, 512], f16, tag="warm")
                nc.gpsimd.memset(warm[:], 0.0)
                psw = psA.tile([128, 512], f32, tag="s1", name="psw")
                for wi in range(8):
                    nc.tensor.matmul(psw[:, 0:512], warm[0:32, 0:128],
                                     warm[0:32, 0:512],
                                     start=(wi == 0), stop=(wi == 7),
                                     skip_group_check=True)
                nc.scalar.copy(wdump[:], psw[:, 0:1])
                it = 0
                for rp in range(3):
                    for gp in range(2):
                        for gh in range(2):
                            g = 2 * gp + gh
                            ps = psA.tile([128, 512], f32, tag="s1")
                            for rr in range(2):
                                rblk = 2 * rp + rr
                                xt = xin[32 * g:32 * (g + 1),
                                         1024 * rblk:1024 * (rblk + 1)]
                                for sp in range(8):
                                    nc.tensor.matmul(
                                        ps[:, 256 * rr + 32 * sp:
                                           256 * rr + 32 * (sp + 1)],
                                        xt[:, 128 * sp:128 * (sp + 1)],
                                        wmisc[32 * g:32 * (g + 1), 0:32],
                                        start=True, stop=True,
                                        tile_position=(32 * g, 0))
                            blk = 12 * gp + 6 * gh + 2 * rp
                            dst = AT[:, 256 * blk:256 * (blk + 2)]
                            if it % 2 == 0:
                                nc.scalar.copy(dst, ps[:])
                            else:
                                nc.vector.tensor_copy(dst, ps[:])
                            it += 1

            atv = AT[:].rearrange("p (s c) -> p s c", s=192, c=32)
            w2v = w2sb[:].rearrange("p (q t k) -> p q t k", q=32, t=4, k=64)

            xin_scope.__exit__(None, None, None)

            # ---- PSUM accumulators (5 banks) ----
            acc_scope = tc.tile_pool(name="acc", bufs=1, space="PSUM")
            accp = acc_scope.__enter__()
            eps0 = accp.tile([128, 480], f32, tag="eps0", name="eps0")
            ops0 = accp.tile([128, 480], f32, tag="ops0", name="ops0")
            eps1 = accp.tile([128, 480], f32, tag="eps1", name="eps1")
            ops1 = accp.tile([128, 480], f32, tag="ops1", name="ops1")
            small = accp.tile([128, 208], f32, tag="small", name="small")
            eps_t = {0: eps0[:], 1: eps1[:], 2: small[:, 0:96]}
            ops_t = {0: ops0[:], 1: ops1[:], 2: small[:, 96:192]}
            zt = small[:, 192:192 + NBLK]
            # bank ids for the start-once bookkeeping; the shared small
            # bank is memset instead (start=True would reset the whole
            # bank and clobber its sibling regions)
            nc.vector.memset(small[:], 0.0)
            started_banks.add("sm")
            eps_bank = {0: "e0", 1: "e1", 2: "sm"}
            ops_bank = {0: "o0", 1: "o1", 2: "sm"}

            psX_scope = tc.tile_pool(name="psX", bufs=2, space="PSUM")
            psX = psX_scope.__enter__()

            with (
                tc.tile_pool(name="phX", bufs=3) as phX,
                tc.tile_pool(name="phT", bufs=3) as phT,
                tc.tile_pool(name="up", bufs=3) as up,
                tc.tile_pool(name="sgp", bufs=2) as sgp,
                tc.tile_pool(name="finp", bufs=2) as finp,
            ):
                def stage2(gi):
                    j0, gw = GRANULES[gi]
                    Xg = phX.tile([128, G_J * 384], f16, tag="Xg",
                                  name=f"Xg{gi}")
                    for jl in range(gw):
                        j = j0 + jl
                        x2 = psX.tile([128, 384], f32, tag="x2")
                        for par in range(2):
                            qv = 2 * j + par
                            qq = qv if qv <= 16 else 32 - qv
                            has_im = 0 < qq < 16
                            are = atv[:, :, qq]
                            re_out = x2[64 * par:64 * (par + 1), 0:192]
                            im_out = x2[64 * par:64 * (par + 1), 192:384]
                            aim = atv[:, :, 16 + qq] if has_im else None
                            nc.tensor.matmul(re_out, w2v[:, qv, 0, :], are,
                                             start=True, stop=not has_im)
                            if has_im:
                                nc.tensor.matmul(re_out, w2v[:, qv, 1, :],
                                                 aim, start=False, stop=True)
                            nc.tensor.matmul(im_out, w2v[:, qv, 2, :], are,
                                             start=True, stop=not has_im)
                            if has_im:
                                nc.tensor.matmul(im_out, w2v[:, qv, 3, :],
                                                 aim, start=False, stop=True)
                        nc.scalar.copy(Xg[:, 384 * jl:384 * (jl + 1)], x2[:])
                    return Xg

                def phat(gi, Xg):
                    j0, gw = GRANULES[gi]
                    xw = Xg[:, 0:gw * 384].rearrange(
                        "p (jl r s) -> p jl r s", jl=gw, r=2, s=192)
                    sq = phT.tile([128, G_J * 384], f16, tag="sq",
                                  name=f"sq{gi}")
                    mg = phT.tile([128, G_J * 192], f16, tag="mg",
                                  name=f"mg{gi}")
                    rb = phT.tile([128, G_J * 192], f16, tag="rb",
                                  name=f"rb{gi}")
                    Ct = phT.tile([128, G_J * 192], f16, tag="Ct",
                                  name=f"Ct{gi}")
                    Dt = phT.tile([128, G_J * 192], f16, tag="Dt",
                                  name=f"Dt{gi}")
                    sqv = sq[:, 0:gw * 384].rearrange(
                        "p (jl r s) -> p jl r s", jl=gw, r=2, s=192)
                    mgv = mg[:, 0:gw * 192].rearrange(
                        "p (jl s) -> p jl s", jl=gw, s=192)
                    nc.scalar.activation(sq[:, 0:gw * 384], Xg[:, 0:gw * 384],
                                         AF.Square)
                    nc.vector.tensor_add(mgv, sqv[:, :, 0, :], sqv[:, :, 1, :])
                    # 1/|X| = exp(-0.5 ln(mag2 + 1e-6)); bias guards f16
                    # underflow (Copy/Square/Ln/Exp share one ACT table)
                    lnt = phT.tile([128, G_J * 192], f16, tag="lnt",
                                   name=f"lnt{gi}")
                    nc.scalar.activation(lnt[:, 0:gw * 192], mg[:, 0:gw * 192],
                                         AF.Ln, bias=bias[:])
                    nc.scalar.activation(rb[:, 0:gw * 192], lnt[:, 0:gw * 192],
                                         AF.Exp, scale=-0.5)
                    rbb = rb[:, 0:gw * 192].rearrange(
                        "p (jl s) -> p jl s", jl=gw, s=192)
                    rbb = rbb.unsqueeze(2).broadcast_to((128, gw, 2, 192))
                    nc.vector.tensor_mul(xw, xw, rbb)
                    av = xw[:, :, 0, :]
                    bv = xw[:, :, 1, :]
                    Cv = Ct[:, 0:gw * 192].rearrange(
                        "p (jl s) -> p jl s", jl=gw, s=192)
                    Dv = Dt[:, 0:gw * 192].rearrange(
                        "p (jl s) -> p jl s", jl=gw, s=192)
                    cd_eng = nc.vector if gi < 2 else nc.gpsimd
                    cd_eng.tensor_add(Cv, av, bv)
                    cd_eng.tensor_sub(Dv, av, bv)
                    return av, bv, Cv, Dv

                POOL_D = (1, 2, 3, 4, 5)

                def zpass(j0, gw, sgv):
                    for jl in range(gw):
                        j = j0 + jl
                        c0col = wmisc[:, 65 + j:66 + j]
                        for bi in range(NBLK):
                            zr0 = 128 * bi
                            zw = min(128, ROWS - zr0)
                            acc_mm("sm", zt[0:zw, bi:bi + 1],
                                   sgv[:, jl, zr0:zr0 + zw], c0col)

                def granule(gi, ph, last):
                    j0, gw = GRANULES[gi]
                    av, bv, Cv, Dv = ph
                    u1 = up.tile([128, G_J * ROWS], f16, tag="u1",
                                 name=f"u1_{gi}")
                    u2 = up.tile([128, G_J * ROWS], f16, tag="u2",
                                 name=f"u2_{gi}")
                    u3 = up.tile([128, G_J * ROWS], f16, tag="u3",
                                 name=f"u3_{gi}")
                    sg = sgp.tile([128, G_J * ROWS], f16, tag="sg",
                                  name=f"sg_{gi}")
                    u1v = u1[:, 0:gw * ROWS].rearrange(
                        "p (jl r) -> p jl r", jl=gw, r=ROWS)
                    u2v = u2[:, 0:gw * ROWS].rearrange(
                        "p (jl r) -> p jl r", jl=gw, r=ROWS)
                    u3v = u3[:, 0:gw * ROWS].rearrange(
                        "p (jl r) -> p jl r", jl=gw, r=ROWS)
                    sgv = sg[:, 0:gw * ROWS].rearrange(
                        "p (jl r) -> p jl r", jl=gw, r=ROWS)

                    def mul_d(uv, lv, rv, d, pool_ok=False):
                        wd = (12 - d) * B
                        o = OFF[d]
                        eng = nc.gpsimd if (pool_ok and d in POOL_D) \
                            else nc.vector
                        eng.tensor_mul(uv[:, :, o:o + wd],
                                       lv[:, :, 0:wd],
                                       rv[:, :, d * B:d * B + wd])

                    def wslice(cs, jl):
                        b0 = 384 * (j0 + jl) + 128 * cs
                        return wcs[:, b0:b0 + 128]

                    def passes(ci, streams, stop_u3=False):
                        c0, cw = CHUNKS[ci]
                        for (kind, uv) in streams:
                            bank = eps_bank[ci] if kind == 0 else ops_bank[ci]
                            acc = eps_t[ci] if kind == 0 else ops_t[ci]
                            for jl in range(gw):
                                acc_mm(bank, acc, wslice(kind, jl),
                                       uv[:, jl, c0:c0 + cw],
                                       stop=(stop_u3 and kind == 1
                                             and uv is u3v
                                             and jl == gw - 1))

                    if not last:
                        # stream-major: muls u1, u2, sg, u3 then PE in the
                        # same production order
                        for d in range(1, 12):
                            mul_d(u1v, av, av, d, pool_ok=(d in U1_POOL))
                        for d in range(1, 12):
                            mul_d(u2v, bv, bv, d)
                        nc.vector.tensor_add(sg[:, 0:gw * ROWS],
                                             u1[:, 0:gw * ROWS],
                                             u2[:, 0:gw * ROWS])
                        for d in range(1, 12):
                            mul_d(u3v, Cv, Dv, d, pool_ok=True)
                        for ci in range(3):
                            passes(ci, [(2, u1v)])
                        for ci in range(3):
                            passes(ci, [(1, u3v)])
                        for ci in range(3):
                            passes(ci, [(1, u2v)])
                        for ci in range(3):
                            passes(ci, [(0, sgv)])
                        zpass(j0, gw, sgv)
                    else:
                        # chunk-major: each chunk finishes (incl nyquist,
                        # combine, DMA out) while the next chunk computes
                        for ci, dlist in [(0, (1, 2, 3)), (1, (4, 5, 6, 7, 8)),
                                          (2, (9, 10, 11))]:
                            c0, cw = CHUNKS[ci]
                            for d in dlist:
                                mul_d(u1v, av, av, d)
                            for d in dlist:
                                mul_d(u2v, bv, bv, d)
                            nc.vector.tensor_add(
                                sgv[:, :, c0:c0 + cw],
                                u1v[:, :, c0:c0 + cw],
                                u2v[:, :, c0:c0 + cw])
                            if ci == 2:
                                zpass(j0, gw, sgv)
                                finish_z()
                            for d in dlist:
                                mul_d(u3v, Cv, Dv, d, pool_ok=True)
                            passes(ci, [(2, u1v), (1, u2v), (0, sgv)])
                            passes(ci, [(1, u3v)], stop_u3=True)
                            finish_chunk(ci)

                def finish_chunk(ci):
                    c0, cw = CHUNKS[ci]
                    # nyquist term joins eps; full-width stop
                    acc_mm(eps_bank[ci], eps_t[ci],
                           cng[0:1, 0:128], cng[0:1, 129 + c0:129 + c0 + cw],
                           stop=True)
                    # outp = eps - ops (lags +1..+128), outm = eps + ops.
                    # TensorTensor may read only ONE input from PSUM:
                    # stage eps to SBUF (ACT) first
                    esb = finp.tile([128, 480], f32, tag="esb",
                                    name=f"esb{ci}")
                    nc.scalar.copy(esb[:, 0:cw], eps_t[ci])
                    nc.vector.tensor_sub(outpsb[:, c0:c0 + cw],
                                         esb[:, 0:cw], ops_t[ci])
                    nc.vector.tensor_add(outmsb[:, c0:c0 + cw],
                                         esb[:, 0:cw], ops_t[ci])
                    nc.sync.dma_start(out=outpd[:, c0:c0 + cw],
                                      in_=outpsb[:, c0:c0 + cw])
                    nc.sync.dma_start(out=outmd[:, c0:c0 + cw],
                                      in_=outmsb[:, c0:c0 + cw])

                def finish_z():
                    for bi in range(NBLK):
                        zr0 = 128 * bi
                        zw = min(128, ROWS - zr0)
                        acc_mm("sm", zt[0:zw, bi:bi + 1],
                               cng[0:1, 129 + zr0:129 + zr0 + zw],
                               cng[0:1, 128:129],
                               stop=(bi == NBLK - 1))
                    nc.vector.tensor_copy(out0sb[:], zt)
                    nc.sync.dma_start(out=out0d[:], in_=out0sb[:])

                # ---- software pipeline, stage2 two granules ahead and
                # phat one ahead: phat(g+1)'s DVE/ACT/Pool ops are issued
                # before muls(g) so the in-order queues can't serialize
                # next-granule prep behind this granule's product burst ----
                NG = len(GRANULES)
                Xgs = {0: stage2(0), 1: stage2(1)}
                phs = {0: phat(0, Xgs.pop(0))}
                for gi in range(NG):
                    if gi + 2 < NG:
                        Xgs[gi + 2] = stage2(gi + 2)
                    if gi + 1 < NG:
                        phs[gi + 1] = phat(gi + 1, Xgs.pop(gi + 1))
                    granule(gi, phs.pop(gi), last=(gi == NG - 1))

            psX_scope.__exit__(None, None, None)
            acc_scope.__exit__(None, None, None)

    _legalize_waits(nc)
    return nc


def pack_inputs_core(xs):
    """xs [16, 12, 4096] f32 -> input map for one core."""
    if "w" not in _COMPILED:
        _COMPILED["w"] = _build_weights()
    W = _COMPILED["w"]
    xdev = xs.astype(np.float16)
    xt1 = xdev.transpose(1, 0, 2).reshape(NS, 32, 128)   # s=(n,b), n1, n2
    xc = xt1.reshape(4, 48, 32, 128).transpose(0, 2, 1, 3) \
        .reshape(128, 48 * 128)
    alt = ((-1.0) ** np.arange(K)).astype(np.float32)
    xnyq = (xs.astype(np.float32) * alt).sum(axis=2)     # [16, 12]
    sgn = np.where(xnyq >= 0, 1.0, -1.0).T               # [12 n, 16 b]
    cng = np.zeros((1, 129 + ROWS), np.float16)
    cng[0, 0:129] = W["cno"]
    for d in range(1, 12):
        for n in range(12 - d):
            r = OFF[d] + n * B
            cng[0, 129 + r:129 + r + B] = \
                (sgn[n] * sgn[n + d]).astype(np.float16)
    return {"x": np.ascontiguousarray(xc), "cng": cng,
            "wmisc": W["wmisc"], "w2d": W["w2d"], "wcs": W["wcs"]}


def unpack_outputs_core(outs, xs):
    """outs dict -> [16, 12, 12, 257] f32."""
    outp = outs["outp"].astype(np.float32)
    outm = outs["outm"].astype(np.float32)
    out0 = outs["out0"]
    zz = np.zeros(ROWS, np.float32)
    for bi in range(NBLK):
        c0 = 128 * bi
        cw = min(128, ROWS - c0)
        zz[c0:c0 + cw] = out0[0:cw, bi]
    blk = np.zeros((B, NSIG, NSIG, 2 * TAU + 1), dtype=np.float32)
    for d in range(1, 12):
        for n in range(12 - d):
            m = n + d
            rows = OFF[d] + n * B + np.arange(B)
            blk[:, n, m, 0] = zz[rows]
            blk[:, n, m, 1:129] = outp[:, rows].T
            blk[:, n, m, 129:] = outm[::-1, rows].T
            blk[:, m, n, 0] = zz[rows]
            blk[:, m, n, 1:] = blk[:, n, m, 1:][:, ::-1]
    for n in range(NSIG):
        blk[:, n, n, 0] = 1.0
    return blk


def _get_compiled():
    if "nc" not in _COMPILED:
        _COMPILED["nc"] = _build_bass()
    return _COMPILED["nc"]


def kernel(x: np.ndarray) -> np.ndarray:
    from concourse.bass_utils import run_bass_kernel_spmd

    nc = _get_compiled()
    x = np.ascontiguousarray(x, dtype=np.float32)
    in_maps = [pack_inputs_core(x[c * B:(c + 1) * B]) for c in range(NCORES)]

    trace = bool(int(os.environ.get("BASS_GCC_TRACE", "0")))
    res = None
    for attempt in range(3):
        try:
            res = run_bass_kernel_spmd(nc, in_maps, list(range(NCORES)),
                                       trace=trace)
            break
        except Exception:
            if attempt == 2:
                raise
            import time
            time.sleep(5)
    _COMPILED["last_result"] = res

    out = np.zeros((NCORES * B, NSIG, NSIG, 2 * TAU + 1), dtype=np.float32)
    for c in range(NCORES):
        out[c * B:(c + 1) * B] = unpack_outputs_core(
            res.results[c], x[c * B:(c + 1) * B])
    return out
